# revision 25
# baseline (speedup 1.0000x reference)
"""Trainium2 Bass kernel for nn_CEKT_18193481466012.

Dataflow analysis of the reference shows the GAT branch (heads, h_cat,
h_gat, adj, W_heads/a_heads/W_out/a_out) never feeds the returned y_t:

    comb  = [e_t, r_t, prev_h]            # [N, 320]
    g     = sigmoid(comb @ Wq_w + Wq_b)   # [N, 128]
    cand  = tanh(comb @ Wh_w + Wh_b)      # [N, 128]
    h_t   = (1 - g) * prev_h + g * cand   # [N, 128]
    comb2 = [h_t, e_t, x]                 # [N, 384]
    y_t   = sigmoid(comb2 @ Wp_w + Wp_b)  # [N, 128]

So the kernel computes exactly that, sharding the N=4096 rows across the
8 NeuronCores (512 rows each). Layout on device is feature-on-partition
(transposed on host), so every matmul contraction chunk is a natural
[K<=128, ...] SBUF tile and the per-output-feature biases are applied as
per-partition activation biases. No collectives: every row's output
depends only on that row's inputs. All arithmetic is fp32 (float32r
would stream 4x faster through the PE but rounds operands to ~11
mantissa bits — measured 1.4e-4 relative error on hardware — which
risks an fp32-referenced correctness check; fp32 lands at ~1e-6).

Raw Bass (no Tile): this walrus build only accepts a single inline
sync-wait per instruction, so cross-engine deps use standalone wait_ge
instructions. Key structure:

 - Per-core inputs are packed host-side into ONE [128, COLS] f32 block,
   DMA'd in four pipelined stages ordered by when each piece is needed
   (PE starts after ~0.33MB instead of 1.5MB). r_t^T [64, 512] is
   repacked as [128, 256] (two column-halves stacked on partitions) so
   no dead partitions are transferred; the [64,128] weight chunks are
   duplicated onto partitions 64:128 so both rt matmuls' operands share
   a base partition.
 - The back end is pipelined in column halves (cand, the h_t DVE chain,
   the final matmul, the y sigmoid, and the output DMA) with separate
   PSUM banks per half, so the ACT read of half 0 never touches the
   bank PE is still accumulating into.
 - A dummy activation preloads the ACT function tables under the DMA.

Semaphores: D1-D4 input DMA stages, DO output DMAs, P PE groups
(pg0=1, pg1=2, pc0=3, pc1=4, py0=5, py1=6), A ACT (g0=1, g1=2,
cand0=3, cand1=4, y0=5, y1=6), V DVE (d0=1, u0=2, ht0=3, d1=4, u1=5,
ht1=6).
"""

import numpy as np

import concourse.bass as bass
from concourse import mybir
from concourse.bass_utils import run_bass_kernel_spmd

N = 4096
NCORES = 8
R = N // NCORES  # 512 rows per core
H = R // 2       # column half

# column offsets in the packed [128, COLS] per-core input block
# stage A1
ET0 = 0           # e_t^T  [128, R]
WQ1 = 512         # Wq_w[0:128]    [128, 128]
A1_END = 640
# stage A2
PH0 = 640         # prev_h^T [128, R]
RT0 = 1152        # r_t^T repacked [128, 256]
WQ2 = 1408        # Wq_w[128:192]  [64, 128], duplicated on partitions 64:128
WQ3 = 1536        # Wq_w[192:320]  [128, 128]
BQ = 1664         # Wq_b [128, 1] column
BH = 1665
BP = 1666
WP1 = 1668        # Wp_w[0:128]    (contracts with h_t)
WP2 = 1796        # Wp_w[128:256]  (contracts with e_t)
A2_END = 1924
# stage A3
WH1 = 1924
WH2 = 2052        # duplicated on partitions 64:128
WH3 = 2180
A3_END = 2308
# stage B
XT0 = 2308        # x^T [128, R]
WP3 = 2820        # Wp_w[256:384]  (contracts with x)
COLS = 2948

_F32 = mybir.dt.float32
_AF = mybir.ActivationFunctionType

_CACHE = {}


def _build_nc():
    nc = bass.Bass()

    pk = nc.dram_tensor("pk", [128, COLS], _F32, kind="ExternalInput")
    outT = nc.dram_tensor("outT", [128, R], _F32, kind="ExternalOutput")

    from contextlib import ExitStack
    stack = ExitStack()
    with stack:
        t = stack.enter_context(nc.sbuf_tensor("t", [128, COLS], _F32))
        g = stack.enter_context(nc.sbuf_tensor("g", [128, R], _F32))
        cand = stack.enter_context(nc.sbuf_tensor("cand", [128, R], _F32))
        d = stack.enter_context(nc.sbuf_tensor("d", [128, R], _F32))
        u = stack.enter_context(nc.sbuf_tensor("u", [128, R], _F32))
        ht = stack.enter_context(nc.sbuf_tensor("ht", [128, R], _F32))
        y = stack.enter_context(nc.sbuf_tensor("y", [128, R], _F32))
        scratch = stack.enter_context(nc.sbuf_tensor("scratch", [128, 1], _F32))
        pg0 = stack.enter_context(nc.psum_tensor("pg0", [128, H], _F32))
        pg1 = stack.enter_context(nc.psum_tensor("pg1", [128, H], _F32))
        pc0 = stack.enter_context(nc.psum_tensor("pc0", [128, H], _F32))
        pc1 = stack.enter_context(nc.psum_tensor("pc1", [128, H], _F32))
        py0 = stack.enter_context(nc.psum_tensor("py0", [128, H], _F32))
        py1 = stack.enter_context(nc.psum_tensor("py1", [128, H], _F32))
        pw = stack.enter_context(nc.psum_tensor("pw", [1, 1], _F32))
        D1 = stack.enter_context(nc.semaphore("D1"))
        D2 = stack.enter_context(nc.semaphore("D2"))
        D3 = stack.enter_context(nc.semaphore("D3"))
        D4 = stack.enter_context(nc.semaphore("D4"))
        DO = stack.enter_context(nc.semaphore("DO"))
        P = stack.enter_context(nc.semaphore("P"))
        A = stack.enter_context(nc.semaphore("A"))
        V = stack.enter_context(nc.semaphore("V"))
        block = stack.enter_context(nc.Block())

        def half(ap, i, width=H):
            return ap[:, i * width:(i + 1) * width]

        et = t[:, ET0:ET0 + R]
        ph = t[:, PH0:PH0 + R]
        xt = t[:, XT0:XT0 + R]
        rta = t[0:64, RT0:RT0 + 256]
        rtb = t[64:128, RT0:RT0 + 256]
        wq1 = t[:, WQ1:WQ1 + 128]
        wq2 = t[0:64, WQ2:WQ2 + 128]
        wq2b = t[64:128, WQ2:WQ2 + 128]
        wq3 = t[:, WQ3:WQ3 + 128]
        wh1 = t[:, WH1:WH1 + 128]
        wh2 = t[0:64, WH2:WH2 + 128]
        wh2b = t[64:128, WH2:WH2 + 128]
        wh3 = t[:, WH3:WH3 + 128]
        wp1 = t[:, WP1:WP1 + 128]
        wp2 = t[:, WP2:WP2 + 128]
        wp3 = t[:, WP3:WP3 + 128]

        @block.sync
        def _(sync):
            sync.dma_start(t[:, 0:A1_END], pk[:, 0:A1_END]).then_inc(D1, 16)
            sync.dma_start(t[:, A1_END:A2_END], pk[:, A1_END:A2_END]).then_inc(D2, 16)
            sync.dma_start(t[:, A2_END:A3_END], pk[:, A2_END:A3_END]).then_inc(D3, 16)
            sync.dma_start(t[:, A3_END:COLS], pk[:, A3_END:COLS]).then_inc(D4, 16)
            # out1 is issued from ACT (see below) so the two output
            # DMAs' HWDGE setup latencies overlap across engines
            sync.wait_ge(A, 5)
            sync.dma_start(outT[:, 0:H], y[:, 0:H]).then_inc(DO, 16)

        @block.tensor
        def _(pe):
            # Every accumulation group is uniformly 256 columns wide: a
            # group whose stop-matmul covers a different column extent
            # than earlier members hangs the PSUM last-write tracking on
            # hardware (verified by bisection).
            # Warm-up matmul on the const tile: moves the PE out of its
            # cold p-state while the input DMA is still in flight.
            zero_pe = nc.const_aps.scalar_like(0.0, scratch[:])
            pe.matmul(pw[:], zero_pe, zero_pe, start=True, stop=True)
            # pg = Wq^T comb^T, one uniform [128,H] group per bank
            pe.wait_ge(D1, 16)
            pe.matmul(pg0[:], wq1, half(et, 0), start=True, stop=False)
            pe.matmul(pg1[:], wq1, half(et, 1), start=True, stop=False)
            pe.wait_ge(D2, 16)
            pe.matmul(pg0[:], wq3, half(ph, 0), start=False, stop=False)
            pe.matmul(pg0[:], wq2, rta, start=False, stop=True).then_inc(P, 1)
            pe.matmul(pg1[:], wq3, half(ph, 1), start=False, stop=False)
            pe.matmul(pg1[:], wq2b, rtb, start=False, stop=True).then_inc(P, 1)
            # pc = Wh^T comb^T
            pe.wait_ge(D3, 16)
            pe.matmul(pc0[:], wh1, half(et, 0), start=True, stop=False)
            pe.matmul(pc0[:], wh3, half(ph, 0), start=False, stop=False)
            pe.matmul(pc0[:], wh2, rta, start=False, stop=True).then_inc(P, 1)
            pe.matmul(pc1[:], wh1, half(et, 1), start=True, stop=False)
            pe.matmul(pc1[:], wh3, half(ph, 1), start=False, stop=False)
            pe.matmul(pc1[:], wh2b, rtb, start=False, stop=True).then_inc(P, 1)
            # py = Wp2^T et + Wp3^T xt + Wp1^T h_t, halved across two banks
            pe.matmul(py0[:], wp2, half(et, 0), start=True, stop=False)
            pe.matmul(py1[:], wp2, half(et, 1), start=True, stop=False)
            pe.wait_ge(D4, 16)
            pe.matmul(py0[:], wp3, half(xt, 0), start=False, stop=False)
            pe.matmul(py1[:], wp3, half(xt, 1), start=False, stop=False)
            pe.wait_ge(V, 3)
            pe.matmul(py0[:], wp1, half(ht, 0), start=False, stop=True).then_inc(P, 1)
            pe.wait_ge(V, 6)
            pe.matmul(py1[:], wp1, half(ht, 1), start=False, stop=True).then_inc(P, 1)

        @block.scalar
        def _(scalar):
            # dummy activation: loads the ACT function tables while the
            # input DMA is still in flight
            zero = nc.const_aps.scalar_like(0.0, scratch[:])
            scalar.activation(scratch[:], zero, _AF.Sigmoid)
            scalar.wait_ge(P, 1)
            scalar.activation(
                g[:, 0:H], pg0[:], _AF.Sigmoid, bias=t[:, BQ:BQ + 1]
            ).then_inc(A, 1)
            scalar.wait_ge(P, 2)
            scalar.activation(
                g[:, H:R], pg1[:], _AF.Sigmoid, bias=t[:, BQ:BQ + 1]
            ).then_inc(A, 1)
            scalar.wait_ge(P, 3)
            scalar.activation(
                cand[:, 0:H], pc0[:], _AF.Tanh, bias=t[:, BH:BH + 1]
            ).then_inc(A, 1)
            scalar.wait_ge(P, 4)
            scalar.activation(
                cand[:, H:R], pc1[:], _AF.Tanh, bias=t[:, BH:BH + 1]
            ).then_inc(A, 1)
            scalar.wait_ge(P, 5)
            scalar.activation(
                y[:, 0:H], py0[:], _AF.Sigmoid, bias=t[:, BP:BP + 1]
            ).then_inc(A, 1)
            scalar.wait_ge(P, 6)
            scalar.activation(
                y[:, H:R], py1[:], _AF.Sigmoid, bias=t[:, BP:BP + 1]
            ).then_inc(A, 1)
            # ACT produced y1 itself; the wait is satisfied immediately
            # (it gates the async DMA read for the race checker)
            scalar.wait_ge(A, 6)
            scalar.dma_start(outT[:, H:R], y[:, H:R]).then_inc(DO, 16)

        @block.vector
        def _(vector):
            # h_t = ph + g*(cand - ph), pipelined in column halves. The
            # DVE pipeline is deep, so same-engine RAW chains need waits.
            vector.wait_ge(A, 3)
            vector.tensor_sub(d[:, 0:H], cand[:, 0:H], ph[:, 0:H]).then_inc(V, 1)
            vector.wait_ge(V, 1)
            vector.tensor_mul(u[:, 0:H], g[:, 0:H], d[:, 0:H]).then_inc(V, 1)
            vector.wait_ge(V, 2)
            vector.tensor_add(ht[:, 0:H], ph[:, 0:H], u[:, 0:H]).then_inc(V, 1)
            vector.wait_ge(A, 4)
            vector.tensor_sub(d[:, H:R], cand[:, H:R], ph[:, H:R]).then_inc(V, 1)
            vector.wait_ge(V, 4)
            vector.tensor_mul(u[:, H:R], g[:, H:R], d[:, H:R]).then_inc(V, 1)
            vector.wait_ge(V, 5)
            vector.tensor_add(ht[:, H:R], ph[:, H:R], u[:, H:R]).then_inc(V, 1)


    return nc


def _get_nc():
    if "nc" not in _CACHE:
        _CACHE["nc"] = _build_nc()
    return _CACHE["nc"]


def _f32(a):
    return np.ascontiguousarray(np.asarray(a), dtype=np.float32)


def _pack(inputs):
    e_t = _f32(inputs["e_t"])
    r_t = _f32(inputs["r_t"])
    prev_h = _f32(inputs["prev_h"])
    x = _f32(inputs["x"])
    Wq = _f32(inputs["Wq_w"])
    Wh = _f32(inputs["Wh_w"])
    Wp = _f32(inputs["Wp_w"])
    bq = _f32(inputs["Wq_b"]).reshape(128)
    bh = _f32(inputs["Wh_b"]).reshape(128)
    bp = _f32(inputs["Wp_b"]).reshape(128)

    packs = []
    for c in range(NCORES):
        rows = slice(c * R, (c + 1) * R)
        p = np.zeros((128, COLS), dtype=np.float32)
        rtT = r_t[rows].T  # [64, 512]
        p[:, ET0:ET0 + R] = e_t[rows].T
        p[:, WQ1:WQ1 + 128] = Wq[0:128]
        p[:, PH0:PH0 + R] = prev_h[rows].T
        p[0:64, RT0:RT0 + 256] = rtT[:, 0:256]
        p[64:128, RT0:RT0 + 256] = rtT[:, 256:512]
        p[0:64, WQ2:WQ2 + 128] = Wq[128:192]
        p[64:128, WQ2:WQ2 + 128] = Wq[128:192]
        p[:, WQ3:WQ3 + 128] = Wq[192:320]
        p[:, BQ] = bq
        p[:, BH] = bh
        p[:, BP] = bp
        p[:, WP1:WP1 + 128] = Wp[0:128]
        p[:, WP2:WP2 + 128] = Wp[128:256]
        p[:, WH1:WH1 + 128] = Wh[0:128]
        p[0:64, WH2:WH2 + 128] = Wh[128:192]
        p[64:128, WH2:WH2 + 128] = Wh[128:192]
        p[:, WH3:WH3 + 128] = Wh[192:320]
        p[:, XT0:XT0 + R] = x[rows].T
        p[:, WP3:WP3 + 128] = Wp[256:384]
        packs.append({"pk": p})
    return packs


def run(inputs, trace=False):
    """Shard, run on cores 0-7, gather. Returns (y [4096,128], results obj)."""
    in_maps = _pack(inputs)
    nc = _get_nc()
    res = run_bass_kernel_spmd(
        nc, in_maps, core_ids=list(range(NCORES)), trace=trace
    )
    y = np.concatenate([res.results[c]["outT"].T for c in range(NCORES)], axis=0)
    return np.ascontiguousarray(y, dtype=np.float32), res


def kernel(**inputs):
    y, _ = run(inputs)
    return y


# revision 30
# speedup vs baseline: 1.0195x; 1.0195x over previous
"""Trainium2 Bass kernel for nn_CEKT_18193481466012.

Dataflow analysis of the reference shows the GAT branch (heads, h_cat,
h_gat, adj, W_heads/a_heads/W_out/a_out) never feeds the returned y_t:

    comb  = [e_t, r_t, prev_h]            # [N, 320]
    g     = sigmoid(comb @ Wq_w + Wq_b)   # [N, 128]
    cand  = tanh(comb @ Wh_w + Wh_b)      # [N, 128]
    h_t   = (1 - g) * prev_h + g * cand   # [N, 128]
    comb2 = [h_t, e_t, x]                 # [N, 384]
    y_t   = sigmoid(comb2 @ Wp_w + Wp_b)  # [N, 128]

So the kernel computes exactly that, sharding the N=4096 rows across the
8 NeuronCores (512 rows each). Layout on device is feature-on-partition
(transposed on host), so every matmul contraction chunk is a natural
[K<=128, ...] SBUF tile and the per-output-feature biases are applied as
per-partition activation biases. No collectives: every row's output
depends only on that row's inputs. All arithmetic is fp32 (float32r
would stream 4x faster through the PE but rounds operands to ~11
mantissa bits — measured 1.4e-4 relative error on hardware — which
risks an fp32-referenced correctness check; fp32 lands at ~1e-6).

Raw Bass (no Tile): this walrus build only accepts a single inline
sync-wait per instruction, so cross-engine deps use standalone wait_ge
instructions. Key structure:

 - Per-core inputs are packed host-side into ONE [128, COLS] f32 block,
   DMA'd in four pipelined stages ordered by when each piece is needed
   (PE starts after ~0.33MB instead of 1.5MB). r_t^T [64, 512] is
   repacked as [128, 256] (two column-halves stacked on partitions) so
   no dead partitions are transferred; the [64,128] weight chunks are
   duplicated onto partitions 64:128 so both rt matmuls' operands share
   a base partition.
 - The back end is pipelined in column halves (cand, the h_t DVE chain,
   the final matmul, the y sigmoid, and the output DMA) with separate
   PSUM banks per half, so the ACT read of half 0 never touches the
   bank PE is still accumulating into.
 - A dummy activation preloads the ACT function tables under the DMA.

Semaphores: D1-D4 input DMA stages, DO output DMAs, P PE groups
(pg0=1, pg1=2, pc1=3, pc0=4, py1=5, py0=6), A ACT (g0=1, g1=2,
cand1=3, cand0=4, y1=5, y0=6), V DVE (d1=1, u1=2, ht1=3, d0=4, u0=5,
ht0=6). Half 1 runs ahead of half 0 end-to-end so the final DMA (half
0, issued by ACT itself) starts as early as possible; a throwaway
48-col matmul burns the PE's cold-p-state window.
"""

import numpy as np

import concourse.bass as bass
from concourse import mybir
from concourse.bass_utils import run_bass_kernel_spmd

N = 4096
NCORES = 8
R = N // NCORES  # 512 rows per core
H = R // 2       # column half
Q = R // 4       # column quarter

# column offsets in the packed [128, COLS] per-core input block
# stage A1
ET0 = 0           # e_t^T  [128, R]
WQ1 = 512         # Wq_w[0:128]    [128, 128]
A1_END = 640
# stage A2
PH0 = 640         # prev_h^T [128, R]
RT0 = 1152        # r_t^T repacked [128, 256]
WQ2 = 1408        # Wq_w[128:192]  [64, 128], duplicated on partitions 64:128
WQ3 = 1536        # Wq_w[192:320]  [128, 128]
BQ = 1664         # Wq_b [128, 1] column
BH = 1665
BP = 1666
WP1 = 1668        # Wp_w[0:128]    (contracts with h_t)
WP2 = 1796        # Wp_w[128:256]  (contracts with e_t)
A2_END = 1924
# stage A3
WH1 = 1924
WH2 = 2052        # duplicated on partitions 64:128
WH3 = 2180
A3_END = 2308
# stage B
XT0 = 2308        # x^T [128, R]
WP3 = 2820        # Wp_w[256:384]  (contracts with x)
COLS = 2948

_F32 = mybir.dt.float32
_AF = mybir.ActivationFunctionType

_CACHE = {}


def _build_nc():
    nc = bass.Bass()

    pk = nc.dram_tensor("pk", [128, COLS], _F32, kind="ExternalInput")
    outT = nc.dram_tensor("outT", [128, R], _F32, kind="ExternalOutput")

    from contextlib import ExitStack
    stack = ExitStack()
    with stack:
        t = stack.enter_context(nc.sbuf_tensor("t", [128, COLS], _F32))
        g = stack.enter_context(nc.sbuf_tensor("g", [128, R], _F32))
        cand = stack.enter_context(nc.sbuf_tensor("cand", [128, R], _F32))
        d = stack.enter_context(nc.sbuf_tensor("d", [128, R], _F32))
        u = stack.enter_context(nc.sbuf_tensor("u", [128, R], _F32))
        ht = stack.enter_context(nc.sbuf_tensor("ht", [128, R], _F32))
        y = stack.enter_context(nc.sbuf_tensor("y", [128, R], _F32))
        scratch = stack.enter_context(nc.sbuf_tensor("scratch", [128, 1], _F32))
        pg0 = stack.enter_context(nc.psum_tensor("pg0", [128, H], _F32))
        pg1 = stack.enter_context(nc.psum_tensor("pg1", [128, H], _F32))
        pc0 = stack.enter_context(nc.psum_tensor("pc0", [128, H], _F32))
        pc1 = stack.enter_context(nc.psum_tensor("pc1", [128, H], _F32))
        py0 = stack.enter_context(nc.psum_tensor("py0", [128, H], _F32))
        py1 = stack.enter_context(nc.psum_tensor("py1", [128, H], _F32))
        D1 = stack.enter_context(nc.semaphore("D1"))
        D2 = stack.enter_context(nc.semaphore("D2"))
        D3 = stack.enter_context(nc.semaphore("D3"))
        D4 = stack.enter_context(nc.semaphore("D4"))
        DO = stack.enter_context(nc.semaphore("DO"))
        P = stack.enter_context(nc.semaphore("P"))
        A = stack.enter_context(nc.semaphore("A"))
        V = stack.enter_context(nc.semaphore("V"))
        block = stack.enter_context(nc.Block())

        def half(ap, i, width=H):
            return ap[:, i * width:(i + 1) * width]

        et = t[:, ET0:ET0 + R]
        ph = t[:, PH0:PH0 + R]
        xt = t[:, XT0:XT0 + R]
        rta = t[0:64, RT0:RT0 + 256]
        rtb = t[64:128, RT0:RT0 + 256]
        wq1 = t[:, WQ1:WQ1 + 128]
        wq2 = t[0:64, WQ2:WQ2 + 128]
        wq2b = t[64:128, WQ2:WQ2 + 128]
        wq3 = t[:, WQ3:WQ3 + 128]
        wh1 = t[:, WH1:WH1 + 128]
        wh2 = t[0:64, WH2:WH2 + 128]
        wh2b = t[64:128, WH2:WH2 + 128]
        wh3 = t[:, WH3:WH3 + 128]
        wp1 = t[:, WP1:WP1 + 128]
        wp2 = t[:, WP2:WP2 + 128]
        wp3 = t[:, WP3:WP3 + 128]

        @block.sync
        def _(sync):
            sync.dma_start(t[:, 0:A1_END], pk[:, 0:A1_END]).then_inc(D1, 16)
            sync.dma_start(t[:, A1_END:A2_END], pk[:, A1_END:A2_END]).then_inc(D2, 16)
            sync.dma_start(t[:, A2_END:A3_END], pk[:, A2_END:A3_END]).then_inc(D3, 16)
            sync.dma_start(t[:, A3_END:COLS], pk[:, A3_END:COLS]).then_inc(D4, 16)
            # half 1 of the output finishes first (pc1 runs before pc0 on
            # PE); half 0 is issued from ACT right after its sigmoid so
            # the final DMA pays no cross-engine sem hop
            sync.wait_ge(A, 5)
            sync.dma_start(outT[:, H:R], y[:, H:R]).then_inc(DO, 16)

        @block.tensor
        def _(pe):
            # Every accumulation group is uniform-width and lives in its
            # own PSUM bank: a group whose stop-matmul covers a different
            # column extent than earlier members hangs the PSUM
            # last-write tracking on hardware (verified by bisection).
            pe.wait_ge(D1, 16)
            # Throwaway 48-col matmul: the PE pays a 2x mid-p-state
            # penalty on matmuls dispatched in its first ~3us; burning
            # that window on 48 columns lets every real matmul dispatch
            # at full clock.
            pe.matmul(pg0[0:1, 0:48], wq1[:, 0:1], et[:, 0:48], start=True, stop=True)
            # pg = Wq^T comb^T, one [128,H] group per bank
            pe.matmul(pg0[:], wq1, half(et, 0), start=True, stop=False)
            pe.matmul(pg1[:], wq1, half(et, 1), start=True, stop=False)
            pe.wait_ge(D2, 16)
            pe.matmul(pg0[:], wq3, half(ph, 0), start=False, stop=False)
            pe.matmul(pg0[:], wq2, rta, start=False, stop=True).then_inc(P, 1)
            pe.matmul(pg1[:], wq3, half(ph, 1), start=False, stop=False)
            pe.matmul(pg1[:], wq2b, rtb, start=False, stop=True).then_inc(P, 1)
            # pc = Wh^T comb^T; half 1 first so the *1 output chain
            # starts as early as possible
            pe.wait_ge(D3, 16)
            pe.matmul(pc1[:], wh1, half(et, 1), start=True, stop=False)
            pe.matmul(pc1[:], wh3, half(ph, 1), start=False, stop=False)
            pe.matmul(pc1[:], wh2b, rtb, start=False, stop=True).then_inc(P, 1)
            pe.matmul(pc0[:], wh1, half(et, 0), start=True, stop=False)
            pe.matmul(pc0[:], wh3, half(ph, 0), start=False, stop=False)
            pe.matmul(pc0[:], wh2, rta, start=False, stop=True).then_inc(P, 1)
            # py = Wp2^T et + Wp3^T xt + Wp1^T h_t, halved across banks;
            # *1 half first
            pe.matmul(py1[:], wp2, half(et, 1), start=True, stop=False)
            pe.matmul(py0[:], wp2, half(et, 0), start=True, stop=False)
            pe.wait_ge(D4, 16)
            pe.matmul(py1[:], wp3, half(xt, 1), start=False, stop=False)
            pe.matmul(py0[:], wp3, half(xt, 0), start=False, stop=False)
            pe.wait_ge(V, 3)
            pe.matmul(py1[:], wp1, half(ht, 1), start=False, stop=True).then_inc(P, 1)
            pe.wait_ge(V, 6)
            pe.matmul(py0[:], wp1, half(ht, 0), start=False, stop=True).then_inc(P, 1)

        @block.scalar
        def _(scalar):
            # dummy activation: loads the ACT function tables while the
            # input DMA is still in flight
            zero = nc.const_aps.scalar_like(0.0, scratch[:])
            scalar.activation(scratch[:], zero, _AF.Sigmoid)
            scalar.wait_ge(P, 1)
            scalar.activation(
                g[:, 0:H], pg0[:], _AF.Sigmoid, bias=t[:, BQ:BQ + 1]
            ).then_inc(A, 1)
            scalar.wait_ge(P, 2)
            scalar.activation(
                g[:, H:R], pg1[:], _AF.Sigmoid, bias=t[:, BQ:BQ + 1]
            ).then_inc(A, 1)
            scalar.wait_ge(P, 3)
            scalar.activation(
                cand[:, H:R], pc1[:], _AF.Tanh, bias=t[:, BH:BH + 1]
            ).then_inc(A, 1)
            scalar.wait_ge(P, 4)
            scalar.activation(
                cand[:, 0:H], pc0[:], _AF.Tanh, bias=t[:, BH:BH + 1]
            ).then_inc(A, 1)
            scalar.wait_ge(P, 5)
            scalar.activation(
                y[:, H:R], py1[:], _AF.Sigmoid, bias=t[:, BP:BP + 1]
            ).then_inc(A, 1)
            scalar.wait_ge(P, 6)
            scalar.activation(
                y[:, 0:H], py0[:], _AF.Sigmoid, bias=t[:, BP:BP + 1]
            ).then_inc(A, 1)
            # ACT produced y half 0 itself; the wait is satisfied
            # immediately (it gates the async DMA read for the checker)
            scalar.wait_ge(A, 6)
            scalar.dma_start(outT[:, 0:H], y[:, 0:H]).then_inc(DO, 16)

        @block.vector
        def _(vector):
            # h_t = ph + g*(cand - ph), half 1 first. The DVE pipeline is
            # deep, so same-engine RAW chains need waits.
            vector.wait_ge(A, 3)
            vector.tensor_sub(d[:, H:R], cand[:, H:R], ph[:, H:R]).then_inc(V, 1)
            vector.wait_ge(V, 1)
            vector.tensor_mul(u[:, H:R], g[:, H:R], d[:, H:R]).then_inc(V, 1)
            vector.wait_ge(V, 2)
            vector.tensor_add(ht[:, H:R], ph[:, H:R], u[:, H:R]).then_inc(V, 1)
            vector.wait_ge(A, 4)
            vector.tensor_sub(d[:, 0:H], cand[:, 0:H], ph[:, 0:H]).then_inc(V, 1)
            vector.wait_ge(V, 4)
            vector.tensor_mul(u[:, 0:H], g[:, 0:H], d[:, 0:H]).then_inc(V, 1)
            vector.wait_ge(V, 5)
            vector.tensor_add(ht[:, 0:H], ph[:, 0:H], u[:, 0:H]).then_inc(V, 1)

    return nc


def _get_nc():
    if "nc" not in _CACHE:
        _CACHE["nc"] = _build_nc()
    return _CACHE["nc"]


def _f32(a):
    return np.ascontiguousarray(np.asarray(a), dtype=np.float32)


def _pack(inputs):
    e_t = _f32(inputs["e_t"])
    r_t = _f32(inputs["r_t"])
    prev_h = _f32(inputs["prev_h"])
    x = _f32(inputs["x"])
    Wq = _f32(inputs["Wq_w"])
    Wh = _f32(inputs["Wh_w"])
    Wp = _f32(inputs["Wp_w"])
    bq = _f32(inputs["Wq_b"]).reshape(128)
    bh = _f32(inputs["Wh_b"]).reshape(128)
    bp = _f32(inputs["Wp_b"]).reshape(128)

    packs = []
    for c in range(NCORES):
        rows = slice(c * R, (c + 1) * R)
        p = np.zeros((128, COLS), dtype=np.float32)
        rtT = r_t[rows].T  # [64, 512]
        p[:, ET0:ET0 + R] = e_t[rows].T
        p[:, WQ1:WQ1 + 128] = Wq[0:128]
        p[:, PH0:PH0 + R] = prev_h[rows].T
        p[0:64, RT0:RT0 + 256] = rtT[:, 0:256]
        p[64:128, RT0:RT0 + 256] = rtT[:, 256:512]
        p[0:64, WQ2:WQ2 + 128] = Wq[128:192]
        p[64:128, WQ2:WQ2 + 128] = Wq[128:192]
        p[:, WQ3:WQ3 + 128] = Wq[192:320]
        p[:, BQ] = bq
        p[:, BH] = bh
        p[:, BP] = bp
        p[:, WP1:WP1 + 128] = Wp[0:128]
        p[:, WP2:WP2 + 128] = Wp[128:256]
        p[:, WH1:WH1 + 128] = Wh[0:128]
        p[0:64, WH2:WH2 + 128] = Wh[128:192]
        p[64:128, WH2:WH2 + 128] = Wh[128:192]
        p[:, WH3:WH3 + 128] = Wh[192:320]
        p[:, XT0:XT0 + R] = x[rows].T
        p[:, WP3:WP3 + 128] = Wp[256:384]
        packs.append({"pk": p})
    return packs


def run(inputs, trace=False):
    """Shard, run on cores 0-7, gather. Returns (y [4096,128], results obj)."""
    in_maps = _pack(inputs)
    nc = _get_nc()
    res = run_bass_kernel_spmd(
        nc, in_maps, core_ids=list(range(NCORES)), trace=trace
    )
    y = np.concatenate([res.results[c]["outT"].T for c in range(NCORES)], axis=0)
    return np.ascontiguousarray(y, dtype=np.float32), res


def kernel(**inputs):
    y, _ = run(inputs)
    return y
